# revision 18
# baseline (speedup 1.0000x reference)
"""Trainium2 Bass kernel for nn_BlockLinear_MixerBlock (6-layer radix-4 butterfly mixer).

Math: 6 block-diagonal butterfly layers (radix 4, gaps 1..1024) over the
feature dim (4096) of x [8192, 4096] compose into a Monarch factorization:
layers 0-2 = dense 64x64 mix within each contiguous 64-chunk (A), layers
3-5 = dense 64x64 mix across stride-64 feature classes (B).

Device dataflow (data-parallel over 8 cores, 1024 batch cols/core), designed
so the PE does ONLY weight-stationary matmuls (no PE transposes at all):

  host:  xT = x.T as f16 [4096, 8192] (feature-major), sliced per core
  A   :  per feature tile t: psA[f',b] = WA_t.T @ xT_t   (lhsT=weights, N=512)
         evict f32->f16 into y1T (feature-major, 64KB/partition tile)
  shuf:  per class-pair m: z_m[64d+c, b] = y1T[64c+2m+d, b]
         -- pure SBUF->SBUF DMA row gather (2KB descriptors), no PE involved
  B   :  psB = WB_m.T @ z_m  (lhsT=weights, N=512), evict f16, DMA out to
         yT[64c''+2m+d, b] rows of the f16 feature-major output
  host:  y = concat(yT).T.astype(f32)

HBM traffic per core: 8.4 MB in + 8.4 MB out (f16); 8.4 MB SBUF<->SBUF
shuffle. PE: 128 matmuls N=512 per 1024 rows (~25 us) -- stays warm (no
transpose-mode ops). Evictions alternate DVE/ACT.

WA/WB are composed on the host from `weights` against the identity in
float64, stored f16 (measured end-to-end rel err ~5e-4, gate is 2e-2).
"""

import numpy as np

import concourse.bass as bass
import concourse.bacc as bacc
import concourse.mybir as mybir
from concourse.tile import TileContext
from concourse.bass_utils import run_bass_kernel_spmd

# ---- problem constants (hardcoded per contract) ----
N_CORES = 8
BS = 8192
D = 4096
BD = 4
NUM_LAYERS = 6
GAPS = [1, 4, 16, 64, 256, 1024]
BPC = BS // N_CORES          # 1024 batch cols per core
NFT = D // 128               # 32 feature tiles

F32 = mybir.dt.float32
F16 = mybir.dt.float16


# ---------------- host-side weight composition ----------------

def _ref_layers(x, weights, layers):
    bs = x.shape[0]
    y = x
    for i in layers:
        gap = GAPS[i]
        y = y.reshape(bs, -1, BD, gap).swapaxes(2, 3)
        y = y.reshape(bs, -1, BD)
        y = np.einsum('bnk,nkm->bnm', y, weights[i])
        y = y.reshape(bs, -1, gap, BD).swapaxes(2, 3)
    return y.reshape(bs, -1)


def _build_stage_weights(weights):
    w64 = weights.astype(np.float64)
    I = np.eye(D, dtype=np.float64)
    MA = _ref_layers(I, w64, [0, 1, 2])   # y1 = x @ MA (block-diag, 64-chunks)
    MB = _ref_layers(I, w64, [3, 4, 5])   # y  = y1 @ MB (block over stride-64)

    WA = np.zeros((128, D), np.float16)
    for t in range(NFT):
        WA[:, 128 * t:128 * (t + 1)] = MA[128 * t:128 * (t + 1), 128 * t:128 * (t + 1)]

    # lhsT for class pair (2m, 2m+1), z row order q = 64d + c:
    #   WB_m[64d + c, 64d + c''] = MB[64c + 2m + d, 64c'' + 2m + d]
    MBr = MB.reshape(64, 64, 64, 64)      # [c, u', c'', u'']
    WB = np.zeros((128, D), np.float16)
    for m in range(NFT):
        for dd in range(2):
            u = 2 * m + dd
            WB[64 * dd:64 * dd + 64, 128 * m + 64 * dd:128 * m + 64 * dd + 64] = \
                MBr[:, u, :, u]
    return WA, WB


# ---------------- device program ----------------

def _build_program(repeats=1, timing_io=False, ablate=()):
    ablate = frozenset(ablate)
    nc = bacc.Bacc("TRN2", target_bir_lowering=False, debug=False)
    if timing_io:
        # timing-only variant: big tensors live in device DRAM (no host I/O)
        xT_d = nc.dram_tensor("xT_int", [D, BPC], F16, kind="Internal")
        yT_d = nc.dram_tensor("yT_int", [D, BPC], F16, kind="Internal")
        yp_d = nc.dram_tensor("yprobe", [128, 4], F16, kind="ExternalOutput")
    else:
        xT_d = nc.dram_tensor("xT", [D, BPC], F16, kind="ExternalInput")
        yT_d = nc.dram_tensor("yT", [D, BPC], F16, kind="ExternalOutput")
        yp_d = None
    wa_d = nc.dram_tensor("wa", [128, D], F16, kind="ExternalInput")
    wb_d = nc.dram_tensor("wb", [128, D], F16, kind="ExternalInput")

    with TileContext(nc) as tc:
        with (
            tc.tile_pool(name="const", bufs=1) as const,
            tc.tile_pool(name="xin", bufs=8) as xin_pool,
            tc.tile_pool(name="y1", bufs=2) as y1_pool,
            tc.tile_pool(name="zbuf", bufs=4) as z_pool,
            tc.tile_pool(name="yout", bufs=4) as yo_pool,
            tc.tile_pool(name="psA", bufs=3, space="PSUM") as psA_pool,
            tc.tile_pool(name="psB", bufs=3, space="PSUM") as psB_pool,
        ):
            # prefetch the first t-quad of x before the 2 MiB of weight DMA so
            # the first matmuls are not stuck behind the weight loads
            x0 = None
            if repeats == 1 and not timing_io:
                x0 = []
                for j in range(4):
                    xt = xin_pool.tile([128, BPC], F16, name="x0", tag="xt")
                    nc.scalar.dma_start(xt[:], xT_d.ap()[128 * j:128 * (j + 1), :])
                    x0.append(xt)
            wa_sb = const.tile([128, D], F16, name="wa_sb")
            wb_sb = const.tile([128, D], F16, name="wb_sb")
            for h in range(8):
                lo, hi = 512 * h, 512 * (h + 1)
                nc.sync.dma_start(wa_sb[:, lo:hi], wa_d.ap()[:, lo:hi])
                nc.sync.dma_start(wb_sb[:, lo:hi], wb_d.ap()[:, lo:hi])

            import contextlib
            if repeats > 1:
                assert repeats % 2 == 0
                # 2x unroll inside the HW loop so the two y1 buffers rotate:
                # iteration i+1's A-phase overlaps iteration i's B-phase
                with tc.For_i(0, repeats // 2, 1):
                    for u in range(2):
                        _body(nc, tc, xT_d, yT_d, wa_sb, wb_sb,
                              xin_pool, y1_pool, z_pool, yo_pool,
                              psA_pool, psB_pool, x0=None, ablate=ablate)
            else:
                _body(nc, tc, xT_d, yT_d, wa_sb, wb_sb,
                      xin_pool, y1_pool, z_pool, yo_pool,
                      psA_pool, psB_pool, x0=x0, ablate=ablate)
            if yp_d is not None:
                probe = const.tile([128, 4], F16, name="probe_sb")
                nc.sync.dma_start(probe[:], yT_d.ap()[0:128, 0:4])
                nc.sync.dma_start(yp_d.ap()[:, :], probe[:])
    nc.compile()
    return nc


def _body(nc, tc, xT_d, yT_d, wa_sb, wb_sb,
          xin_pool, y1_pool, z_pool, yo_pool, psA_pool, psB_pool, x0=None,
          ablate=frozenset()):

    def do_copy(k, out_ap, in_ap):
        # alternate PSUM-eviction copies between DVE and ACT
        if k % 2 == 0:
            nc.vector.tensor_copy(out_ap, in_ap)
        else:
            nc.scalar.copy(out_ap, in_ap)

    do_amm = "no_amm" not in ablate
    do_xpose = "no_shuf" not in ablate and do_amm
    do_bmm = "no_bmm" not in ablate and do_amm

    # y1b: b-major stage-A output, 8 tiles [128 b, 4096] f16, cols class-
    # grouped: col = 64*u' + c  (c = 2t + e)
    y1b = [y1_pool.tile([128, D], F16, name="y1b", tag=f"y1b{bb}")
           for bb in range(8)]

    # ---- phase A: per t-quad g: load 4 xt tiles, 8x(4 matmuls -> 1 evict) ----
    for g in range(8):
        xts = []
        for j in range(4):
            t = 4 * g + j
            if x0 is not None and g == 0:
                xts.append(x0[j])
                continue
            xt = xin_pool.tile([128, BPC], F16, name="xt", tag="xt")
            if "no_in" not in ablate:
                nc.scalar.dma_start(xt[:], xT_d.ap()[128 * t:128 * (t + 1), :])
            xts.append(xt)
        if not do_amm:
            continue
        for bb in range(8):
            psA = psA_pool.tile([128, 512], F32, name="psA", tag="psA")
            for j in range(4):
                t = 4 * g + j
                nc.tensor.matmul(
                    psA[:, 128 * j:128 * (j + 1)],
                    lhsT=xts[j][:, 128 * bb:128 * (bb + 1)],
                    rhs=wa_sb[:, 128 * t:128 * (t + 1)],
                    start=True, stop=True,
                )
            # psA col = 128j + 64e + u'  ->  y1b[bb] col = 64u' + (8g + 2j + e)
            srcv = psA[:].rearrange("b (j e u) -> b u (j e)", j=4, e=2)
            dstv = y1b[bb][:].rearrange("b (u c) -> b u c", c=64)[:, :, 8 * g:8 * g + 8]
            do_copy(8 * g + bb, dstv, srcv)

    # ---- phase B: per class-pair m: 8 xbar transposes -> 2 matmuls -> out ----
    yTv = yT_d.ap().rearrange("(c u) b -> u c b", u=64)
    for m in range(NFT):
        z = None
        if do_xpose:
            # z[64d + c, 128bb + b] = y1b[bb][b, 128m + 64d + c]
            z = z_pool.tile([128, BPC], F16, name="z", tag="z")
            for bb in range(8):
                nc.sync.dma_start(z[:, 128 * bb:128 * (bb + 1)],
                                  y1b[bb][:, 128 * m:128 * (m + 1)], transpose=True)
        yo = None
        if do_bmm:
            yo = yo_pool.tile([128, BPC], F16, name="yo", tag="yo")
            rhs_t = z if z is not None else y1b[0]
            for h in range(2):
                psB = psB_pool.tile([128, 512], F32, name="psB", tag="psB")
                nc.tensor.matmul(
                    psB[:],
                    lhsT=wb_sb[:, 128 * m:128 * (m + 1)],
                    rhs=rhs_t[:, 512 * h:512 * (h + 1)],
                    start=True, stop=True,
                )
                do_copy(2 * m + h + 1, yo[:, 512 * h:512 * (h + 1)], psB[:])
        for dd in range(2 if "no_out" not in ablate else 0):
            # yT[64c'' + 2m + d, b] = yo[64d + c'', b]
            dst = yTv[2 * m + dd:2 * m + dd + 1].squeeze()  # [c'':64, b]
            osrc = yo if yo is not None else wb_sb
            nc.gpsimd.dma_start(dst, osrc[64 * dd:64 * dd + 64, 0:BPC])


_PROGRAMS = {}


def _get_program(repeats=1):
    if repeats not in _PROGRAMS:
        _PROGRAMS[repeats] = _build_program(repeats)
    return _PROGRAMS[repeats]


def _run(x, weights, repeats=1, **spmd_kwargs):
    assert x.shape == (BS, D), x.shape
    WA, WB = _build_stage_weights(np.asarray(weights, dtype=np.float32))
    xT = np.ascontiguousarray(np.asarray(x, dtype=np.float16).T)   # [D, BS]
    nc = _get_program(repeats)
    in_maps = [
        {
            "xT": np.ascontiguousarray(xT[:, c * BPC:(c + 1) * BPC]),
            "wa": WA,
            "wb": WB,
        }
        for c in range(N_CORES)
    ]
    res = run_bass_kernel_spmd(nc, in_maps, core_ids=list(range(N_CORES)), **spmd_kwargs)
    yT = np.concatenate([res.results[c]["yT"] for c in range(N_CORES)], axis=1)
    return np.ascontiguousarray(yT.T).astype(np.float32), res


def kernel(x, weights):
    y, _ = _run(x, weights)
    return y


def _run_timing(weights, repeats, n_calls=6):
    """Delta-timing helper: runs the internal-I/O variant; returns wall times."""
    import time
    WA, WB = _build_stage_weights(np.asarray(weights, dtype=np.float32))
    key = ("timing", repeats)
    if key not in _PROGRAMS:
        _PROGRAMS[key] = _build_program(repeats, timing_io=True)
    nc = _PROGRAMS[key]
    in_maps = [{"wa": WA, "wb": WB} for _ in range(N_CORES)]
    walls = []
    for _ in range(n_calls):
        t0 = time.time()
        run_bass_kernel_spmd(nc, in_maps, core_ids=list(range(N_CORES)))
        walls.append(time.time() - t0)
    return walls


# revision 19
# speedup vs baseline: 6.3731x; 6.3731x over previous
"""Trainium2 Bass kernel for nn_BlockLinear_MixerBlock (6-layer radix-4 butterfly mixer).

Math: 6 block-diagonal butterfly layers (radix 4, gaps 1..1024) over the
feature dim (4096) of x [8192, 4096] compose into a Monarch factorization:
layers 0-2 = dense 64x64 mix within each contiguous 64-chunk (A), layers
3-5 = dense 64x64 mix across stride-64 feature classes (B).

Device dataflow (data-parallel over 8 cores, 1024 batch cols/core), designed
so the PE does ONLY weight-stationary matmuls (no PE transposes at all):

  host:  xT = x.T as f16 [4096, 8192] (feature-major), sliced per core
  A   :  per feature tile t: psA[f',b] = WA_t.T @ xT_t   (lhsT=weights, N=512)
         evict f32->f16 into y1T (feature-major, 64KB/partition tile)
  shuf:  per class-pair m: z_m[64d+c, b] = y1T[64c+2m+d, b]
         -- pure SBUF->SBUF DMA row gather (2KB descriptors), no PE involved
  B   :  psB = WB_m.T @ z_m  (lhsT=weights, N=512), evict f16, DMA out to
         yT[64c''+2m+d, b] rows of the f16 feature-major output
  host:  y = concat(yT).T.astype(f32)

HBM traffic per core: 8.4 MB in + 8.4 MB out (f16); 8.4 MB SBUF<->SBUF
shuffle. PE: 128 matmuls N=512 per 1024 rows (~25 us) -- stays warm (no
transpose-mode ops). Evictions alternate DVE/ACT.

WA/WB are composed on the host from `weights` against the identity in
float64, stored f16 (measured end-to-end rel err ~5e-4, gate is 2e-2).
"""

import numpy as np

import concourse.bass as bass
import concourse.bacc as bacc
import concourse.mybir as mybir
from concourse.tile import TileContext
from concourse.bass_utils import run_bass_kernel_spmd

# ---- problem constants (hardcoded per contract) ----
N_CORES = 8
BS = 8192
D = 4096
BD = 4
NUM_LAYERS = 6
GAPS = [1, 4, 16, 64, 256, 1024]
BPC = BS // N_CORES          # 1024 batch cols per core
NFT = D // 128               # 32 feature tiles

F32 = mybir.dt.float32
F16 = mybir.dt.float16


# ---------------- host-side weight composition ----------------

def _ref_layers(x, weights, layers):
    bs = x.shape[0]
    y = x
    for i in layers:
        gap = GAPS[i]
        y = y.reshape(bs, -1, BD, gap).swapaxes(2, 3)
        y = y.reshape(bs, -1, BD)
        y = np.einsum('bnk,nkm->bnm', y, weights[i])
        y = y.reshape(bs, -1, gap, BD).swapaxes(2, 3)
    return y.reshape(bs, -1)


def _build_stage_weights(weights):
    w64 = weights.astype(np.float64)
    I = np.eye(D, dtype=np.float64)
    MA = _ref_layers(I, w64, [0, 1, 2])   # y1 = x @ MA (block-diag, 64-chunks)
    MB = _ref_layers(I, w64, [3, 4, 5])   # y  = y1 @ MB (block over stride-64)

    WA = np.zeros((128, D), np.float16)
    for t in range(NFT):
        WA[:, 128 * t:128 * (t + 1)] = MA[128 * t:128 * (t + 1), 128 * t:128 * (t + 1)]

    # lhsT for class pair (2m, 2m+1), z row order q = 64d + c:
    #   WB_m[64d + c, 64d + c''] = MB[64c + 2m + d, 64c'' + 2m + d]
    MBr = MB.reshape(64, 64, 64, 64)      # [c, u', c'', u'']
    WB = np.zeros((128, D), np.float16)
    for m in range(NFT):
        for dd in range(2):
            u = 2 * m + dd
            WB[64 * dd:64 * dd + 64, 128 * m + 64 * dd:128 * m + 64 * dd + 64] = \
                MBr[:, u, :, u]
    return WA, WB


# ---------------- device program ----------------

def _build_program(repeats=1, timing_io=False, ablate=()):
    ablate = frozenset(ablate)
    nc = bacc.Bacc("TRN2", target_bir_lowering=False, debug=False)
    if timing_io:
        # timing-only variant: big tensors live in device DRAM (no host I/O)
        xT_d = nc.dram_tensor("xT_int", [D, BPC], F16, kind="Internal")
        yT_d = nc.dram_tensor("yT_int", [D, BPC], F16, kind="Internal")
        yp_d = nc.dram_tensor("yprobe", [128, 4], F16, kind="ExternalOutput")
    else:
        xT_d = nc.dram_tensor("xT", [D, BPC], F16, kind="ExternalInput")
        yT_d = nc.dram_tensor("yT", [D, BPC], F16, kind="ExternalOutput")
        yp_d = None
    wa_d = nc.dram_tensor("wa", [128, D], F16, kind="ExternalInput")
    wb_d = nc.dram_tensor("wb", [128, D], F16, kind="ExternalInput")
    id_d = nc.dram_tensor("ident", [128, 128], F16, kind="ExternalInput")

    with TileContext(nc) as tc:
        with (
            tc.tile_pool(name="const", bufs=1) as const,
            tc.tile_pool(name="xin", bufs=8) as xin_pool,
            tc.tile_pool(name="y1", bufs=2) as y1_pool,
            tc.tile_pool(name="zbuf", bufs=4) as z_pool,
            tc.tile_pool(name="yout", bufs=4) as yo_pool,
            tc.tile_pool(name="psA", bufs=3, space="PSUM") as psA_pool,
            tc.tile_pool(name="psB", bufs=2, space="PSUM") as psB_pool,
            tc.tile_pool(name="psT", bufs=3, space="PSUM") as psT_pool,
        ):
            # prefetch the first t-quad of x before the 2 MiB of weight DMA so
            # the first matmuls are not stuck behind the weight loads
            x0 = None
            if repeats == 1 and not timing_io:
                x0 = []
                for j in range(4):
                    xt = xin_pool.tile([128, BPC], F16, name="x0", tag="xt")
                    nc.sync.dma_start(xt[:], xT_d.ap()[128 * j:128 * (j + 1), :])
                    x0.append(xt)
            ident16 = const.tile([128, 128], F16, name="ident16")
            nc.sync.dma_start(ident16[:], id_d.ap())
            wa_sb = const.tile([128, D], F16, name="wa_sb")
            wb_sb = const.tile([128, D], F16, name="wb_sb")
            for h in range(8):
                lo, hi = 512 * h, 512 * (h + 1)
                nc.sync.dma_start(wa_sb[:, lo:hi], wa_d.ap()[:, lo:hi])
                nc.sync.dma_start(wb_sb[:, lo:hi], wb_d.ap()[:, lo:hi])

            import contextlib
            if repeats > 1:
                assert repeats % 2 == 0
                # 2x unroll inside the HW loop so the two y1 buffers rotate:
                # iteration i+1's A-phase overlaps iteration i's B-phase
                with tc.For_i(0, repeats // 2, 1):
                    for u in range(2):
                        _body(nc, tc, xT_d, yT_d, wa_sb, wb_sb, ident16,
                              xin_pool, y1_pool, z_pool, yo_pool,
                              psA_pool, psB_pool, psT_pool, x0=None,
                              ablate=ablate)
            else:
                _body(nc, tc, xT_d, yT_d, wa_sb, wb_sb, ident16,
                      xin_pool, y1_pool, z_pool, yo_pool,
                      psA_pool, psB_pool, psT_pool, x0=x0, ablate=ablate)
            if yp_d is not None:
                probe = const.tile([128, 4], F16, name="probe_sb")
                nc.sync.dma_start(probe[:], yT_d.ap()[0:128, 0:4])
                nc.sync.dma_start(yp_d.ap()[:, :], probe[:])
    nc.compile()
    return nc


def _body(nc, tc, xT_d, yT_d, wa_sb, wb_sb, ident16,
          xin_pool, y1_pool, z_pool, yo_pool, psA_pool, psB_pool, psT_pool,
          x0=None, ablate=frozenset()):

    def do_copy(k, out_ap, in_ap):
        # alternate PSUM-eviction copies between DVE and ACT
        if k % 2 == 0:
            nc.vector.tensor_copy(out_ap, in_ap)
        else:
            nc.scalar.copy(out_ap, in_ap)

    do_amm = "no_amm" not in ablate
    do_xpose = "no_shuf" not in ablate and do_amm
    do_bmm = "no_bmm" not in ablate and do_amm

    # y1b: b-major stage-A output, 8 tiles [128 b, 4096] f16, cols class-
    # grouped: col = 64*u' + c  (c = 2t + e)
    y1b = [y1_pool.tile([128, D], F16, name="y1b", tag=f"y1b{bb}")
           for bb in range(8)]

    # ---- phase A: per t-quad g: load 4 xt tiles, 8x(4 matmuls -> 1 evict) ----
    for g in range(8):
        xts = []
        for j in range(4):
            t = 4 * g + j
            if x0 is not None and g == 0:
                xts.append(x0[j])
                continue
            xt = xin_pool.tile([128, BPC], F16, name="xt", tag="xt")
            if "no_in" not in ablate:
                nc.sync.dma_start(xt[:], xT_d.ap()[128 * t:128 * (t + 1), :])
            xts.append(xt)
        if not do_amm:
            continue
        for bb in range(8):
            psA = psA_pool.tile([128, 512], F32, name="psA", tag="psA")
            for j in range(4):
                t = 4 * g + j
                nc.tensor.matmul(
                    psA[:, 128 * j:128 * (j + 1)],
                    lhsT=xts[j][:, 128 * bb:128 * (bb + 1)],
                    rhs=wa_sb[:, 128 * t:128 * (t + 1)],
                    start=True, stop=True,
                )
            # psA col = 128j + 64e + u'  ->  y1b[bb] col = 64u' + (8g + 2j + e)
            srcv = psA[:].rearrange("b (j e u) -> b u (j e)", j=4, e=2)
            dstv = y1b[bb][:].rearrange("b (u c) -> b u c", c=64)[:, :, 8 * g:8 * g + 8]
            do_copy(8 * g + bb, dstv, srcv)

    # ---- phase B: per class-pair m: 8 PE transposes (2 quads) -> 2 matmuls ----
    # out rows stored device-order: yT_dev[128m + 64d + c''] = y[.., 64c''+2m+d]
    # (host un-permutes); out-DMA is one contiguous 256 KB store per m.
    for m in range(NFT):
        z = None
        if do_xpose:
            # z[64d + c, 128bb + b] = y1b[bb][b, 128m + 64d + c]
            z = z_pool.tile([128, BPC], F16, name="z", tag="z")
            for h in range(2):
                psT = psT_pool.tile([128, 512], F16, name="psT", tag="psT")
                for j in range(4):
                    bb = 4 * h + j
                    nc.tensor.transpose(
                        psT[:, 128 * j:128 * (j + 1)],
                        y1b[bb][:, 128 * m:128 * (m + 1)],
                        ident16[:],
                    )
                do_copy(2 * m + h, z[:, 512 * h:512 * (h + 1)], psT[:])
        yo = None
        if do_bmm:
            yo = yo_pool.tile([128, BPC], F16, name="yo", tag="yo")
            rhs_t = z if z is not None else y1b[0]
            for h in range(2):
                psB = psB_pool.tile([128, 512], F32, name="psB", tag="psB")
                nc.tensor.matmul(
                    psB[:],
                    lhsT=wb_sb[:, 128 * m:128 * (m + 1)],
                    rhs=rhs_t[:, 512 * h:512 * (h + 1)],
                    start=True, stop=True,
                )
                do_copy(2 * m + h + 1, yo[:, 512 * h:512 * (h + 1)], psB[:])
        if "no_out" not in ablate:
            osrc = yo if yo is not None else wb_sb[:, 0:BPC]
            nc.gpsimd.dma_start(yT_d.ap()[128 * m:128 * (m + 1), :], osrc[:])


_PROGRAMS = {}


def _get_program(repeats=1):
    if repeats not in _PROGRAMS:
        _PROGRAMS[repeats] = _build_program(repeats)
    return _PROGRAMS[repeats]


def _run(x, weights, repeats=1, **spmd_kwargs):
    assert x.shape == (BS, D), x.shape
    WA, WB = _build_stage_weights(np.asarray(weights, dtype=np.float32))
    xT = np.ascontiguousarray(np.asarray(x, dtype=np.float16).T)   # [D, BS]
    nc = _get_program(repeats)
    ident = np.eye(128, dtype=np.float16)
    in_maps = [
        {
            "xT": np.ascontiguousarray(xT[:, c * BPC:(c + 1) * BPC]),
            "wa": WA,
            "wb": WB,
            "ident": ident,
        }
        for c in range(N_CORES)
    ]
    res = run_bass_kernel_spmd(nc, in_maps, core_ids=list(range(N_CORES)), **spmd_kwargs)
    yT_dev = np.concatenate([res.results[c]["yT"] for c in range(N_CORES)], axis=1)
    # device row 128m + 64d + c''  ->  true row 64c'' + 2m + d
    yT = yT_dev.reshape(NFT, 2, 64, BS).transpose(2, 0, 1, 3).reshape(D, BS)
    return np.ascontiguousarray(yT.T).astype(np.float32), res


def kernel(x, weights):
    y, _ = _run(x, weights)
    return y


def _run_timing(weights, repeats, n_calls=6):
    """Delta-timing helper: runs the internal-I/O variant; returns wall times."""
    import time
    WA, WB = _build_stage_weights(np.asarray(weights, dtype=np.float32))
    key = ("timing", repeats)
    if key not in _PROGRAMS:
        _PROGRAMS[key] = _build_program(repeats, timing_io=True)
    nc = _PROGRAMS[key]
    ident = np.eye(128, dtype=np.float16)
    in_maps = [{"wa": WA, "wb": WB, "ident": ident} for _ in range(N_CORES)]
    walls = []
    for _ in range(n_calls):
        t0 = time.time()
        run_bass_kernel_spmd(nc, in_maps, core_ids=list(range(N_CORES)))
        walls.append(time.time() - t0)
    return walls


# revision 23
# speedup vs baseline: 6.9626x; 1.0925x over previous
"""Trainium2 Bass kernel for nn_BlockLinear_MixerBlock (6-layer radix-4 butterfly mixer).

Math: 6 block-diagonal butterfly layers (radix 4, gaps 1..1024) over the
feature dim (4096) of x [8192, 4096] compose into a Monarch factorization:
layers 0-2 = dense 64x64 mix within each contiguous 64-chunk (A), layers
3-5 = dense 64x64 mix across stride-64 feature classes (B).

Device dataflow (data-parallel over 8 cores, 1024 batch cols/core), designed
so the PE does ONLY weight-stationary matmuls (no PE transposes at all):

  host:  xT = x.T as f16 [4096, 8192] (feature-major), sliced per core
  A   :  per feature tile t: psA[f',b] = WA_t.T @ xT_t   (lhsT=weights, N=512)
         evict f32->f16 into y1T (feature-major, 64KB/partition tile)
  shuf:  per class-pair m: z_m[64d+c, b] = y1T[64c+2m+d, b]
         -- pure SBUF->SBUF DMA row gather (2KB descriptors), no PE involved
  B   :  psB = WB_m.T @ z_m  (lhsT=weights, N=512), evict f16, DMA out to
         yT[64c''+2m+d, b] rows of the f16 feature-major output
  host:  y = concat(yT).T.astype(f32)

HBM traffic per core: 8.4 MB in + 8.4 MB out (f16); 8.4 MB SBUF<->SBUF
shuffle. PE: 128 matmuls N=512 per 1024 rows (~25 us) -- stays warm (no
transpose-mode ops). Evictions alternate DVE/ACT.

WA/WB are composed on the host from `weights` against the identity in
float64, stored f16 (measured end-to-end rel err ~5e-4, gate is 2e-2).
"""

import numpy as np

import concourse.bass as bass
import concourse.bacc as bacc
import concourse.mybir as mybir
from concourse.tile import TileContext
from concourse.bass_utils import run_bass_kernel_spmd

# ---- problem constants (hardcoded per contract) ----
N_CORES = 8
BS = 8192
D = 4096
BD = 4
NUM_LAYERS = 6
GAPS = [1, 4, 16, 64, 256, 1024]
BPC = BS // N_CORES          # 1024 batch cols per core
NFT = D // 128               # 32 feature tiles

F32 = mybir.dt.float32
F16 = mybir.dt.float16


# ---------------- host-side weight composition ----------------

def _ref_layers(x, weights, layers):
    bs = x.shape[0]
    y = x
    for i in layers:
        gap = GAPS[i]
        y = y.reshape(bs, -1, BD, gap).swapaxes(2, 3)
        y = y.reshape(bs, -1, BD)
        y = np.einsum('bnk,nkm->bnm', y, weights[i])
        y = y.reshape(bs, -1, gap, BD).swapaxes(2, 3)
    return y.reshape(bs, -1)


def _build_stage_weights(weights):
    w64 = weights.astype(np.float64)
    I = np.eye(D, dtype=np.float64)
    MA = _ref_layers(I, w64, [0, 1, 2])   # y1 = x @ MA (block-diag, 64-chunks)
    MB = _ref_layers(I, w64, [3, 4, 5])   # y  = y1 @ MB (block over stride-64)

    WA = np.zeros((128, D), np.float16)
    for t in range(NFT):
        WA[:, 128 * t:128 * (t + 1)] = MA[128 * t:128 * (t + 1), 128 * t:128 * (t + 1)]

    # lhsT for class pair (2m, 2m+1), z row order q = 64d + c:
    #   WB_m[64d + c, 64d + c''] = MB[64c + 2m + d, 64c'' + 2m + d]
    MBr = MB.reshape(64, 64, 64, 64)      # [c, u', c'', u'']
    WB = np.zeros((128, D), np.float16)
    for m in range(NFT):
        for dd in range(2):
            u = 2 * m + dd
            WB[64 * dd:64 * dd + 64, 128 * m + 64 * dd:128 * m + 64 * dd + 64] = \
                MBr[:, u, :, u]
    return WA, WB


# ---------------- device program ----------------

def _build_program(repeats=1, timing_io=False, ablate=()):
    ablate = frozenset(ablate)
    nc = bacc.Bacc("TRN2", target_bir_lowering=False, debug=False)
    if timing_io:
        # timing-only variant: big tensors live in device DRAM (no host I/O)
        xT_d = nc.dram_tensor("xT_int", [D, BPC], F16, kind="Internal")
        yT_d = nc.dram_tensor("yT_int", [D, BPC], F16, kind="Internal")
        yp_d = nc.dram_tensor("yprobe", [128, 4], F16, kind="ExternalOutput")
    else:
        xT_d = nc.dram_tensor("xT", [D, BPC], F16, kind="ExternalInput")
        yT_d = nc.dram_tensor("yT", [D, BPC], F16, kind="ExternalOutput")
        yp_d = None
    wa_d = nc.dram_tensor("wa", [128, D], F16, kind="ExternalInput")
    wb_d = nc.dram_tensor("wb", [128, D], F16, kind="ExternalInput")
    id_d = nc.dram_tensor("ident", [128, 128], F16, kind="ExternalInput")

    with TileContext(nc) as tc:
        with (
            tc.tile_pool(name="const", bufs=1) as const,
            tc.tile_pool(name="xin", bufs=3) as xin_pool,
            tc.tile_pool(name="y1", bufs=2) as y1_pool,
            tc.tile_pool(name="zbuf", bufs=4) as z_pool,
            tc.tile_pool(name="yout", bufs=2) as yo_pool,
            tc.tile_pool(name="psA", bufs=3, space="PSUM") as psA_pool,
            tc.tile_pool(name="psB", bufs=2, space="PSUM") as psB_pool,
            tc.tile_pool(name="psT", bufs=3, space="PSUM") as psT_pool,
        ):
            # prefetch the first t-quad of x before the 2 MiB of weight DMA so
            # the first matmuls are not stuck behind the weight loads
            x0 = None
            if repeats == 1 and not timing_io:
                x0 = xin_pool.tile([128, 4 * BPC], F16, name="x0", tag="xt")
                nc.sync.dma_start(
                    x0[:].rearrange("p (j b) -> p j b", j=4),
                    xT_d.ap().rearrange("(g j p) b -> g p j b", g=8, j=4)[0:1].squeeze())
            ident16 = const.tile([128, 128], F16, name="ident16")
            nc.sync.dma_start(ident16[:], id_d.ap())
            wa_sb = const.tile([128, D], F16, name="wa_sb")
            wb_sb = const.tile([128, D], F16, name="wb_sb")
            for h in range(8):
                lo, hi = 512 * h, 512 * (h + 1)
                nc.sync.dma_start(wa_sb[:, lo:hi], wa_d.ap()[:, lo:hi])
                nc.sync.dma_start(wb_sb[:, lo:hi], wb_d.ap()[:, lo:hi])

            import contextlib
            if repeats > 1:
                assert repeats % 2 == 0
                # 2x unroll inside the HW loop so the two y1 buffers rotate:
                # iteration i+1's A-phase overlaps iteration i's B-phase
                with tc.For_i(0, repeats // 2, 1):
                    for u in range(2):
                        _body(nc, tc, xT_d, yT_d, wa_sb, wb_sb, ident16,
                              xin_pool, y1_pool, z_pool, yo_pool,
                              psA_pool, psB_pool, psT_pool, x0=None,
                              ablate=ablate)
            else:
                _body(nc, tc, xT_d, yT_d, wa_sb, wb_sb, ident16,
                      xin_pool, y1_pool, z_pool, yo_pool,
                      psA_pool, psB_pool, psT_pool, x0=x0, ablate=ablate)
            if yp_d is not None:
                probe = const.tile([128, 4], F16, name="probe_sb")
                nc.sync.dma_start(probe[:], yT_d.ap()[0:128, 0:4])
                nc.sync.dma_start(yp_d.ap()[:, :], probe[:])
    nc.compile()
    return nc


def _body(nc, tc, xT_d, yT_d, wa_sb, wb_sb, ident16,
          xin_pool, y1_pool, z_pool, yo_pool, psA_pool, psB_pool, psT_pool,
          x0=None, ablate=frozenset()):

    def do_copy(k, out_ap, in_ap):
        # alternate PSUM-eviction copies between DVE and ACT
        if k % 2 == 0:
            nc.vector.tensor_copy(out_ap, in_ap)
        else:
            nc.scalar.copy(out_ap, in_ap)

    do_amm = "no_amm" not in ablate
    do_xpose = "no_shuf" not in ablate and do_amm
    do_bmm = "no_bmm" not in ablate and do_amm

    # y1b: b-major stage-A output, 8 tiles [128 b, 4096] f16, cols class-
    # grouped: col = 64*u' + c  (c = 2t + e)
    y1b = [y1_pool.tile([128, D], F16, name="y1b", tag=f"y1b{bb}")
           for bb in range(8)]

    # ---- phase A: per t-quad g: one 1 MiB load, 8x(4 matmuls -> 1 evict) ----
    # xq layout [128 p, 4 j, 1024 b]: xT row 128(4g+j) + p at col 1024j + b
    xTq = xT_d.ap().rearrange("(g j p) b -> g p j b", g=8, j=4)
    for g in range(8):
        if x0 is not None and g == 0:
            xq = x0
        elif "no_in" in ablate:
            xq = wa_sb
        else:
            xq = xin_pool.tile([128, 4 * BPC], F16, name="xq", tag="xt")
            nc.sync.dma_start(
                xq[:].rearrange("p (j b) -> p j b", j=4),
                xTq[g:g + 1].squeeze())
        if not do_amm:
            continue
        for bb in range(8):
            psA = psA_pool.tile([128, 512], F32, name="psA", tag="psA")
            for j in range(4):
                t = 4 * g + j
                nc.tensor.matmul(
                    psA[:, 128 * j:128 * (j + 1)],
                    lhsT=xq[:, BPC * j + 128 * bb:BPC * j + 128 * (bb + 1)],
                    rhs=wa_sb[:, 128 * t:128 * (t + 1)],
                    start=True, stop=True,
                )
            # psA col = 128j + 64e + u'  ->  y1b[bb] col = 64u' + (8g + 2j + e)
            srcv = psA[:].rearrange("b (j e u) -> b u (j e)", j=4, e=2)
            dstv = y1b[bb][:].rearrange("b (u c) -> b u c", c=64)[:, :, 8 * g:8 * g + 8]
            do_copy(8 * g + bb, dstv, srcv)

    # ---- phase B: per class-pair m: 8 PE transposes (2 quads) -> 2 matmuls ----
    # 1-m skew: T(m+1) is emitted before B(m) so the PE never stalls on the
    # psT->z eviction. Out rows stored device-order (host un-permutes):
    # yT_dev[128m + 64d + c''] = y[.., 64c''+2m+d]; one 1 MiB store per 4 m.
    def emit_T(m):
        z = z_pool.tile([128, BPC], F16, name="z", tag="z")
        for h in range(2):
            psT = psT_pool.tile([128, 512], F16, name="psT", tag="psT")
            for j in range(4):
                bb = 4 * h + j
                nc.tensor.transpose(
                    psT[:, 128 * j:128 * (j + 1)],
                    y1b[bb][:, 128 * m:128 * (m + 1)],
                    ident16[:],
                )
            do_copy(2 * m + h, z[:, 512 * h:512 * (h + 1)], psT[:])
        return z

    def emit_B(m, z, yo4):
        rhs_t = z if z is not None else y1b[0]
        for h in range(2):
            psB = psB_pool.tile([128, 512], F32, name="psB", tag="psB")
            nc.tensor.matmul(
                psB[:],
                lhsT=wb_sb[:, 128 * m:128 * (m + 1)],
                rhs=rhs_t[:, 512 * h:512 * (h + 1)],
                start=True, stop=True,
            )
            if yo4 is not None:
                do_copy(2 * m + h + 1,
                        yo4[:, BPC * (m % 4) + 512 * h:BPC * (m % 4) + 512 * (h + 1)],
                        psB[:])

    yo_map = {}

    def get_yo(m):
        g = m // 4
        if g not in yo_map:
            yo_map[g] = yo_pool.tile([128, 4 * BPC], F16, name="yo4", tag="yo")
        return yo_map[g]

    def flush(g):
        osrc = yo_map[g] if g in yo_map else wb_sb
        nc.gpsimd.dma_start(
            yT_d.ap()[512 * g:512 * (g + 1), :].rearrange("(q p) b -> p q b", q=4),
            osrc[:].rearrange("p (q b) -> p q b", q=4))

    z_prev = None
    for m in range(NFT):
        z = emit_T(m) if do_xpose else None
        if m > 0 and do_bmm:
            emit_B(m - 1, z_prev, get_yo(m - 1))
        z_prev = z
        prev = m - 1
        if prev >= 0 and prev % 4 == 3 and "no_out" not in ablate:
            flush(prev // 4)
    if do_bmm:
        emit_B(NFT - 1, z_prev, get_yo(NFT - 1))
    if "no_out" not in ablate:
        flush(7)


_PROGRAMS = {}


def _get_program(repeats=1):
    if repeats not in _PROGRAMS:
        _PROGRAMS[repeats] = _build_program(repeats)
    return _PROGRAMS[repeats]


def _run(x, weights, repeats=1, **spmd_kwargs):
    assert x.shape == (BS, D), x.shape
    WA, WB = _build_stage_weights(np.asarray(weights, dtype=np.float32))
    xT = np.ascontiguousarray(np.asarray(x, dtype=np.float16).T)   # [D, BS]
    nc = _get_program(repeats)
    ident = np.eye(128, dtype=np.float16)
    in_maps = [
        {
            "xT": np.ascontiguousarray(xT[:, c * BPC:(c + 1) * BPC]),
            "wa": WA,
            "wb": WB,
            "ident": ident,
        }
        for c in range(N_CORES)
    ]
    res = run_bass_kernel_spmd(nc, in_maps, core_ids=list(range(N_CORES)), **spmd_kwargs)
    yT_dev = np.concatenate([res.results[c]["yT"] for c in range(N_CORES)], axis=1)
    # device row 128m + 64d + c''  ->  true row 64c'' + 2m + d
    yT = yT_dev.reshape(NFT, 2, 64, BS).transpose(2, 0, 1, 3).reshape(D, BS)
    return np.ascontiguousarray(yT.T).astype(np.float32), res


def kernel(x, weights):
    y, _ = _run(x, weights)
    return y


def _run_timing(weights, repeats, n_calls=6):
    """Delta-timing helper: runs the internal-I/O variant; returns wall times."""
    import time
    WA, WB = _build_stage_weights(np.asarray(weights, dtype=np.float32))
    key = ("timing", repeats)
    if key not in _PROGRAMS:
        _PROGRAMS[key] = _build_program(repeats, timing_io=True)
    nc = _PROGRAMS[key]
    ident = np.eye(128, dtype=np.float16)
    in_maps = [{"wa": WA, "wb": WB, "ident": ident} for _ in range(N_CORES)]
    walls = []
    for _ in range(n_calls):
        t0 = time.time()
        run_bass_kernel_spmd(nc, in_maps, core_ids=list(range(N_CORES)))
        walls.append(time.time() - t0)
    return walls
